# revision 13
# baseline (speedup 1.0000x reference)
"""ConvNeXt-parallel-SSM block, fully on-device Trainium2 Bass kernel.

Math: the FFT depthwise conv + frequency-domain SSM scan collapse into a
single per-channel circular convolution with combined spectrum
G = dw_f * B_f * sum_{t<8} A_f^t.  The kernel computes, per core (one batch
sample each across 8 cores):

  h   = irfft2-style( rfft2(x) * G )        # 4 DFT matmul stages on PE
  hn  = layernorm(h)                        # stats on DVE, affine folded
  y   = gelu(hn @ W1' + b1') @ W2' + b2' + x

2D DFT is done as matmuls with DFT matrices, exploiting conjugate symmetry
(29 of 56 H-freqs kept), block-diagonal K-packing so the 128-row PE array is
fully used, and complex arithmetic folded into stationary matrices.  The two
partition<->free transposes between DFT stages bounce through DRAM in bf16.
"""
import os
import numpy as np
import ml_dtypes
import concourse.bacc as bacc
import concourse.mybir as mybir
import concourse.tile as tile
from concourse.bass_utils import run_bass_kernel_spmd

BN, HH, WW, CC = 8, 56, 56, 192
KF = 29          # rfft rows kept (H-freqs 0..28)
HID = 768
EPS = 1e-6
T_STEPS = 8
f32 = mybir.dt.float32
f32r = mybir.dt.float32r
bf16 = mybir.dt.bfloat16
BF = ml_dtypes.bfloat16
AX = mybir.AxisListType
ALU = mybir.AluOpType
ACTF = mybir.ActivationFunctionType

_CACHE = {}

# column chunking: A/D have 5376 = 14*384 columns, B/C have 5568 = 14*384+192
CH = 384
NA = 14
B_CHUNKS = [(i * CH, min(CH, 5568 - i * CH)) for i in range(15)]


def _build_nc():
    nc = bacc.Bacc("TRN2", target_bir_lowering=False, debug=False,
                   enable_asserts=False, num_devices=8)
    ap = {}
    ap["x"] = nc.dram_tensor("x", [HH, WW, CC], f32, kind="ExternalInput").ap()
    ap["gr"] = nc.dram_tensor("gr", [56, KF * CC], bf16, kind="ExternalInput").ap()
    ap["gi"] = nc.dram_tensor("gi", [56, KF * CC], bf16, kind="ExternalInput").ap()
    ap["ast"] = nc.dram_tensor("ast", [112, 116], f32, kind="ExternalInput").ap()
    ap["bst"] = nc.dram_tensor("bst", [120, 120], bf16, kind="ExternalInput").ap()
    ap["cst"] = nc.dram_tensor("cst", [120, 120], bf16, kind="ExternalInput").ap()
    ap["cst2"] = nc.dram_tensor("cst2", [120, 120], bf16, kind="ExternalInput").ap()
    ap["dstm"] = nc.dram_tensor("dstm", [116, 112], bf16, kind="ExternalInput").ap()
    ap["idm"] = nc.dram_tensor("idm", [112, 112], f32, kind="ExternalInput").ap()
    ap["w1a"] = nc.dram_tensor("w1a", [128, 6, 128], bf16, kind="ExternalInput").ap()
    ap["w1b"] = nc.dram_tensor("w1b", [64, 6, 128], bf16, kind="ExternalInput").ap()
    ap["b1v"] = nc.dram_tensor("b1v", [128, 6], f32, kind="ExternalInput").ap()
    ap["w2"] = nc.dram_tensor("w2", [128, 6, CC], bf16, kind="ExternalInput").ap()
    ap["b2v"] = nc.dram_tensor("b2v", [CC], f32, kind="ExternalInput").ap()
    ap["y"] = nc.dram_tensor("y", [HH, WW, CC], f32, kind="ExternalOutput").ap()
    ud = nc.dram_tensor("ud", [58, 56, CC], bf16, kind="Internal").ap()
    td = nc.dram_tensor("td", [2, 56, KF, CC], bf16, kind="Internal").ap()
    td_flat = td.rearrange("r m k c -> (r m) (k c)")

    with tile.TileContext(nc) as tc:
        with (
            tc.tile_pool(name="const", bufs=1) as const,
            tc.tile_pool(name="work", bufs=3) as work,
            tc.tile_pool(name="ps", bufs=2, space="PSUM") as ps,
        ):
            # ---- constants / weights ----
            ast_sb = const.tile([112, 116], f32r, tag="ast")
            nc.sync.dma_start(out=ast_sb[:], in_=ap["ast"].bitcast(f32r))
            bst_sb = const.tile([120, 120], bf16, tag="bst")
            nc.sync.dma_start(out=bst_sb[:], in_=ap["bst"])
            cst_sb = const.tile([120, 120], bf16, tag="cst")
            nc.sync.dma_start(out=cst_sb[:], in_=ap["cst"])
            cst2_sb = const.tile([120, 120], bf16, tag="cst2")
            nc.sync.dma_start(out=cst2_sb[:], in_=ap["cst2"])
            dst_sb = const.tile([116, 112], bf16, tag="dstm")
            nc.sync.dma_start(out=dst_sb[:], in_=ap["dstm"])
            id_sb = const.tile([112, 112], f32, tag="idm")
            nc.sync.dma_start(out=id_sb[:], in_=ap["idm"])
            g1_sb = const.tile([120, 5568], bf16, tag="g1")
            nc.vector.memset(g1_sb[:], 0.0)
            nc.sync.dma_start(out=g1_sb[0:56, :], in_=ap["gr"])
            nc.sync.dma_start(out=g1_sb[64:120, :], in_=ap["gr"])
            g2_sb = const.tile([120, 5568], bf16, tag="g2")
            nc.vector.memset(g2_sb[:], 0.0)
            nc.sync.dma_start(out=g2_sb[0:56, :], in_=ap["gi"])
            nc.sync.dma_start(out=g2_sb[64:120, :], in_=ap["gi"])
            w1a_sb = const.tile([128, 6, 128], bf16, tag="w1a")
            nc.sync.dma_start(out=w1a_sb[:], in_=ap["w1a"])
            w1b_sb = const.tile([64, 6, 128], bf16, tag="w1b")
            nc.sync.dma_start(out=w1b_sb[:], in_=ap["w1b"])
            b1_sb = const.tile([128, 6], f32, tag="b1v")
            nc.sync.dma_start(out=b1_sb[:], in_=ap["b1v"])
            w2_sb = const.tile([128, 6, CC], bf16, tag="w2")
            nc.sync.dma_start(out=w2_sb[:], in_=ap["w2"])
            b2_sb = const.tile([112, CC], f32, tag="b2v")
            nc.sync.dma_start(out=b2_sb[:],
                              in_=ap["b2v"].unsqueeze(0).broadcast_to([112, CC]))

            # ---- x loads ----
            # xA: partitions 0..55 = rows h (w<28), 56..111 = rows h (w>=28)
            xa_sb = const.tile([112, 5376], f32r, tag="big32")
            nc.sync.dma_start(
                out=xa_sb[0:56, :],
                in_=ap["x"][:, 0:28, :].rearrange("n j c -> n (j c)").bitcast(f32r))
            nc.sync.dma_start(
                out=xa_sb[56:112, :],
                in_=ap["x"][:, 28:56, :].rearrange("n j c -> n (j c)").bitcast(f32r))
            # xtok: partition = (half, n) token row, free = (j, c)
            xt_sb = const.tile([112, 28, CC], f32, tag="xtok")
            nc.sync.dma_start(out=xt_sb[0:56, :, :], in_=ap["x"][:, 0:28, :])
            nc.sync.dma_start(out=xt_sb[56:112, :, :], in_=ap["x"][:, 28:56, :])

            # ---- stage A: DFT over H (contract h).  out rows: (kr29,ki29) x halves
            u_sb = const.tile([116, 5376], bf16, tag="u_t")
            for i in range(NA):
                sl = slice(i * CH, (i + 1) * CH)
                pa = ps.tile([116, CH], f32, tag="mma")
                nc.tensor.matmul(pa[:, :], ast_sb[:], xa_sb[:, sl],
                                 start=True, stop=True)
                nc.vector.tensor_copy(out=u_sb[:, sl], in_=pa[:, :])
                # T1 write: (k, w-pair, c) slices of ud
                nc.sync.dma_start(
                    out=ud[:, 2 * i:2 * i + 2, :],
                    in_=u_sb[0:58, sl].rearrange("k (j c) -> k j c", c=CC))
                nc.sync.dma_start(
                    out=ud[:, 28 + 2 * i:28 + 2 * i + 2, :],
                    in_=u_sb[58:116, sl].rearrange("k (j c) -> k j c", c=CC))

            # ---- stage B/C: DFT over W, pointwise G, inverse DFT over W ----
            up_sb = const.tile([120, 5568], bf16, tag="up")
            nc.vector.memset(up_sb[:], 0.0)
            v_sb = const.tile([120, 5568], bf16, tag="v_tpp")
            x_pw = const.tile([120, 5568], bf16, tag="xpw")
            y_pw = const.tile([120, 5568], bf16, tag="ypw")
            t_sb = const.tile([120, 5568], bf16, tag="u_t")
            for j, (off, n) in enumerate(B_CHUNKS):
                sl = slice(off, off + n)
                nk = n // CC  # k-groups in this chunk (2 or 1)
                ks = slice(2 * j, 2 * j + nk)
                # T1 read: U' rows = (w | real, w | imag), cols = (k, c)
                nc.sync.dma_start(
                    out=up_sb[0:56, sl].rearrange("w (k c) -> w k c", c=CC),
                    in_=ud[ks, :, :].transpose([1, 0, 2]))
                nc.sync.dma_start(
                    out=up_sb[64:120, sl].rearrange("w (k c) -> w k c", c=CC),
                    in_=ud[29 + 2 * j:29 + 2 * j + nk, :, :].transpose([1, 0, 2]))
                pb = ps.tile([120, CH], f32, tag="mmb")
                nc.tensor.matmul(pb[:, 0:n], bst_sb[:], up_sb[:, sl],
                                 start=True, stop=True)
                nc.scalar.copy(out=v_sb[:, sl], in_=pb[:, 0:n])
                # pointwise complex multiply by G
                nc.vector.tensor_mul(x_pw[:, sl], v_sb[:, sl], g1_sb[:, sl])
                nc.gpsimd.tensor_mul(y_pw[:, sl], v_sb[:, sl], g2_sb[:, sl])
                # complex combine folded into C: T = Cst^T X + Cst2^T Y
                pc = ps.tile([120, CH], f32, tag="mmc")
                nc.tensor.matmul(pc[:, 0:n], cst_sb[:], x_pw[:, sl],
                                 start=True, stop=False)
                nc.tensor.matmul(pc[:, 0:n], cst2_sb[:], y_pw[:, sl],
                                 start=False, stop=True)
                nc.scalar.copy(out=t_sb[:, sl], in_=pc[:, 0:n])
                nc.sync.dma_start(out=td_flat[0:56, sl], in_=t_sb[0:56, sl])
                nc.sync.dma_start(out=td_flat[56:112, sl], in_=t_sb[64:120, sl])

            # ---- stage D: inverse DFT over H + LN stats ----
            tpp_sb = const.tile([116, 5376], bf16, tag="v_tpp")
            hp_sb = const.tile([112, 5376], f32, tag="big32")
            susum = const.tile([112, 28], f32, tag="susum")
            sqsum = const.tile([112, 28], f32, tag="sqsum")
            for t in range(NA):
                sl = slice(t * CH, (t + 1) * CH)
                ms = slice(2 * t, 2 * t + 2)
                for blk, (ri, moff) in enumerate([(0, 0), (1, 0), (0, 28), (1, 28)]):
                    nc.sync.dma_start(
                        out=tpp_sb[29 * blk:29 * blk + 29, sl]
                            .rearrange("k (j c) -> k j c", c=CC),
                        in_=td[ri, moff + 2 * t:moff + 2 * t + 2, :, :]
                            .transpose([1, 0, 2]))
                pd = ps.tile([112, CH], f32, tag="mma")
                nc.tensor.matmul(pd[:, :], dst_sb[:], tpp_sb[:, sl],
                                 start=True, stop=True)
                nc.scalar.copy(out=hp_sb[:, sl], in_=pd[:, :])
                h3 = hp_sb[:, sl].rearrange("p (m c) -> p m c", c=CC)
                nc.vector.tensor_reduce(susum[:, ms], h3, axis=AX.X, op=ALU.add)
                sq = work.tile([112, CH], f32, tag="sqch")
                nc.gpsimd.tensor_mul(sq[:, :], hp_sb[:, sl], hp_sb[:, sl])
                nc.vector.tensor_reduce(sqsum[:, ms],
                                        sq[:, :].rearrange("p (m c) -> p m c", c=CC),
                                        axis=AX.X, op=ALU.add)

            # ---- LN finalize ----
            mu = const.tile([112, 28], f32, tag="mu")
            nc.scalar.mul(out=mu[:], in_=susum[:], mul=1.0 / CC)
            musq = const.tile([112, 28], f32, tag="musq")
            nc.scalar.square(out=musq[:], in_=mu[:])
            var_t = const.tile([112, 28], f32, tag="var")
            nc.vector.scalar_tensor_tensor(out=var_t[:], in0=sqsum[:], scalar=1.0 / CC,
                                           in1=musq[:], op0=ALU.mult, op1=ALU.subtract)
            eps_sb = const.tile([112, 1], f32, tag="eps")
            nc.vector.memset(eps_sb[:], EPS)
            std = const.tile([112, 28], f32, tag="std")
            nc.scalar.activation(out=std[:], in_=var_t[:], func=ACTF.Sqrt,
                                 bias=eps_sb[:], scale=1.0)
            rstd = const.tile([112, 28], f32, tag="rstd")
            nc.vector.reciprocal(out=rstd[:], in_=std[:])
            h3 = hp_sb[:, :].rearrange("p (m c) -> p m c", c=CC)
            mub = mu[:].unsqueeze(2).broadcast_to([112, 28, CC])
            rsb = rstd[:].unsqueeze(2).broadcast_to([112, 28, CC])
            nc.vector.tensor_sub(h3, h3, mub)
            nc.vector.tensor_mul(h3, h3, rsb)

            # ---- transposes to [c, token] (token = (j, half, n)) ----
            hn1 = const.tile([128, 3136], bf16, tag="hn1")
            hn2 = const.tile([64, 3136], bf16, tag="hn2")
            for j in range(28):
                ts = slice(j * 112, (j + 1) * 112)
                pt1 = ps.tile([128, 112], f32, tag="mma")
                nc.tensor.matmul(pt1[:, :],
                                 hp_sb[:, :].rearrange("p (m c) -> p m c", c=CC)
                                 [:, j, 0:128],
                                 id_sb[:], is_transpose=True,
                                 start=True, stop=True)
                nc.scalar.copy(out=hn1[:, ts], in_=pt1[:, :])
                pt2 = ps.tile([64, 112], f32, tag="mmc")
                nc.tensor.matmul(pt2[:, :],
                                 hp_sb[:, :].rearrange("p (m c) -> p m c", c=CC)
                                 [:, j, 128:192],
                                 id_sb[:], is_transpose=True,
                                 start=True, stop=True)
                nc.scalar.copy(out=hn2[:, ts], in_=pt2[:, :])

            # ---- MLP ----
            for q in range(7):
                qs = slice(q * 448, (q + 1) * 448)
                g_sb = work.tile([128, 6, 448], bf16, tag="g")
                for j in range(6):
                    pm = ps.tile([128, 448], f32, tag="mmb")
                    nc.tensor.matmul(pm[:, :], w1a_sb[:, j, :], hn1[:, qs],
                                     start=True, stop=False)
                    nc.tensor.matmul(pm[:, :], w1b_sb[:, j, :], hn2[:, qs],
                                     start=False, stop=True)
                    nc.scalar.activation(out=g_sb[:, j, :], in_=pm[:, :],
                                         func=ACTF.Gelu_apprx_tanh,
                                         bias=b1_sb[:, j:j + 1], scale=1.0)
                for tq in range(4):
                    tt = 4 * q + tq
                    py = ps.tile([112, CC], f32, tag="y")
                    for j in range(6):
                        nc.tensor.matmul(py[:, :], g_sb[:, j, 112 * tq:112 * (tq + 1)],
                                         w2_sb[:, j, :], start=(j == 0), stop=(j == 5))
                    ysb = work.tile([112, CC], f32, tag="ysb")
                    nc.vector.tensor_add(ysb[:, :], py[:, :], xt_sb[:, tt, :])
                    nc.vector.tensor_add(ysb[:, :], ysb[:, :], b2_sb[:, :])
                    nc.sync.dma_start(out=ap["y"][:, tt, :], in_=ysb[0:56, :])
                    nc.sync.dma_start(out=ap["y"][:, 28 + tt, :], in_=ysb[56:112, :])
    nc.compile()
    return nc


def _host_prep(dw_kernel, A_kernel, B_kernel, ln_scale, ln_bias, W1, b1, W2, b2, gamma):
    def pad_kernel(kernel):
        Cc, k, _ = kernel.shape
        c = k // 2
        out = np.zeros((Cc, HH, WW), np.float32)
        for i in range(k):
            for jj in range(k):
                out[:, (i - c) % HH, (jj - c) % WW] = kernel[:, i, jj]
        return out

    def kfft(kernel):
        # rfft over H (axis 1), full fft over W (axis 2) -> (C, 29, 56)
        return np.fft.fft(np.fft.rfft(pad_kernel(kernel), axis=1), axis=2)

    dw_f = kfft(np.asarray(dw_kernel, np.float32))
    A_f = kfft((0.9 * np.tanh(np.asarray(A_kernel, np.float64))).astype(np.float32))
    B_f = kfft(np.asarray(B_kernel, np.float32))
    S = np.ones_like(A_f)
    P = np.ones_like(A_f)
    for _ in range(1, T_STEPS):
        P = P * A_f
        S = S + P
    G = dw_f * B_f * S                          # (C, 29k, 56l)
    G_lkc = G.transpose(2, 1, 0)                # (l, k, c)
    gr = np.ascontiguousarray(G_lkc.real.astype(BF)).reshape(56, KF * CC)
    gi = np.ascontiguousarray(G_lkc.imag.astype(BF)).reshape(56, KF * CC)

    hh = np.arange(HH)
    kk = np.arange(KF)
    th = 2 * np.pi * np.outer(hh, kk) / HH
    C58 = np.concatenate([np.cos(th), -np.sin(th)], axis=1).astype(np.float32)
    ast = np.zeros((112, 116), np.float32)
    ast[0:56, 0:58] = C58
    ast[56:112, 58:116] = C58

    ll = np.arange(WW)
    twl = 2 * np.pi * np.outer(ll, ll) / WW
    cos_wl = np.cos(twl).astype(np.float32)
    sin_wl = np.sin(twl).astype(np.float32)
    bst = np.zeros((120, 120), np.float32)
    bst[0:56, 0:56] = cos_wl
    bst[64:120, 0:56] = sin_wl
    bst[0:56, 64:120] = -sin_wl
    bst[64:120, 64:120] = cos_wl
    cst = np.zeros((120, 120), np.float32)
    cst[0:56, 0:56] = cos_wl
    cst[64:120, 0:56] = -sin_wl
    cst[0:56, 64:120] = sin_wl
    cst[64:120, 64:120] = cos_wl
    cst2 = np.zeros((120, 120), np.float32)
    cst2[0:56, :] = cst[64:120, :]
    cst2[64:120, :] = -cst[0:56, :]

    wk = np.where((kk == 0) | (kk == KF - 1), 1.0, 2.0) / (HH * WW)
    tnk = 2 * np.pi * np.outer(kk, hh) / HH
    Dblk = np.concatenate([wk[:, None] * np.cos(tnk),
                           -wk[:, None] * np.sin(tnk)], axis=0).astype(np.float32)
    dstm = np.zeros((116, 112), np.float32)
    dstm[0:58, 0:56] = Dblk
    dstm[58:116, 56:112] = Dblk

    W1f = (np.asarray(ln_scale, np.float64)[:, None] * np.asarray(W1, np.float64))
    b1f = (np.asarray(ln_bias, np.float64) @ np.asarray(W1, np.float64)
           + np.asarray(b1, np.float64)).astype(np.float32)
    W2g = (np.asarray(W2, np.float64) * np.asarray(gamma, np.float64)[None, :])
    b2g = (np.asarray(gamma, np.float64) * np.asarray(b2, np.float64)).astype(np.float32)

    # W1f[c, hid]: hid block j covers cols 128j..128j+127 -> [c, j, m] row-major
    w1a = np.ascontiguousarray(W1f[0:128].astype(BF).reshape(128, 6, 128))
    w1b = np.ascontiguousarray(W1f[128:192].astype(BF).reshape(64, 6, 128))
    b1v = np.ascontiguousarray(b1f.reshape(6, 128).T)
    w2 = np.ascontiguousarray(
        W2g.astype(BF).reshape(6, 128, CC).transpose(1, 0, 2))
    return dict(gr=gr, gi=gi, ast=ast, bst=bst.astype(BF), cst=cst.astype(BF),
                cst2=cst2.astype(BF),
                dstm=dstm.astype(BF), idm=np.eye(112, dtype=np.float32),
                w1a=w1a, w1b=w1b, b1v=b1v, w2=w2, b2v=b2g)


def kernel(x, dw_kernel, A_kernel, B_kernel, ln_scale, ln_bias, W1, b1, W2, b2, gamma):
    wts = _host_prep(dw_kernel, A_kernel, B_kernel, ln_scale, ln_bias,
                     W1, b1, W2, b2, gamma)
    if "nc" not in _CACHE:
        _CACHE["nc"] = _build_nc()
    nc = _CACHE["nc"]
    x = np.ascontiguousarray(np.asarray(x, np.float32))
    in_maps = []
    for b in range(BN):
        m = {"x": x[b]}
        m.update(wts)
        in_maps.append(m)
    trace = bool(os.environ.get("BASS_KERNEL_TRACE"))
    res = run_bass_kernel_spmd(nc, in_maps, list(range(BN)), trace=trace)
    if trace:
        _CACHE["exec_ns"] = res.exec_time_ns
        _CACHE["profile"] = res.profile_json
        _CACHE["res"] = res
    out = np.empty((BN, HH, WW, CC), np.float32)
    for b in range(BN):
        out[b] = res.results[b]["y"]
    return out
